# revision 55
# baseline (speedup 1.0000x reference)
"""CombinePatches (3D col2im fold + overlap-count normalize) on 8 TRN2 NeuronCores.

Decomposition (validated numerically against the reference):
  out[b, 2q+kd, 2s+kh, 2u+kw, c] (+)= patches[b, q, s, u, kd, kh, kw, c], then
  out /= cnt, cnt = cd(d)*ch(h)*cw(w) separable overlap counts.

Sharding: 8 cores = B(2) x D-chunks(4). Each core computes 16 output d-rows
from 9 od-slices of patches (1 halo slice, zero-padded at global edges by the
host). Everything moves as bf16 (input staging, matmul, output); the 2e-2
rel-err budget dwarfs the ~3e-3 this costs, and it halves the HBM traffic
that bounds this kernel (memory regime).

Per core the DRAM staging is [p=(uhalf,s) 126 rows][vpair][kd][j][x][t][c]:
  - one HWDGE load per slice (single DMA; 126 rows -> 14 SDMA engines,
    dodging the chronically slow engine 15),
  - DVE kw-fold: ONE fully contiguous add per slice (vp0 half + vp1 half),
  - DVE kd-fold: ONE contiguous 2048-elem add builds both rows' j-major
    matmul rhs at once (kd0/kd1 planes of slice k + kd2/kd3 of slice k-1),
  - TensorE h-fold: O[h, (w,c)] = sum_j Mh_j^T @ T_j, 8 matmuls/row (K=126)
    into a 2-bank PSUM tile; 0.25*rh(h) baked into the block-diagonal Mh,
  - ScalarE eviction PSUM -> bf16 SBUF, one 2-row store on the scalar ring.
Host fixes the global d-edge rows and w-edge columns by x2 after gather.
"""
import sys

for _p in ("/opt/trn_rl_repo", "/opt/trn_rl_repo/pypackages"):
    if _p not in sys.path:
        sys.path.insert(0, _p)

from contextlib import ExitStack

import numpy as np

import concourse.bass as bass
import concourse.tile as tile
from concourse import bacc, mybir
from concourse import bass_utils

B, D, H, W, C = 2, 64, 128, 128, 4
od, oh, ow = 31, 63, 63
NS = 9              # od-slices per core (incl 1 halo)
RPC = 16            # output d-rows per core
MM_DT = mybir.dt.bfloat16
import ml_dtypes

BF16 = ml_dtypes.bfloat16

# per-partition free width of a slice with nkd kd-planes:
# [vpair=2][kd=nkd][j=4][x=32][t=2][c=4] -- vpair outermost, so the whole
# kw-fold of a slice is ONE fully contiguous DVE add (vp0 half + vp1 half),
# and each folded kd-plane is a contiguous j-major 1024-elem matmul rhs.
def _fw(nkd):
    return 2 * nkd * 1024


# 126 data partitions: p<63 = (uhalf 0, s=p), 63<=p<126 = (uhalf 1, s=p-63).
# The two all-zero s=63 pad rows are never transferred; they would land on
# partitions served by SDMA engine 15, which is ~20% slower than the rest
# and was the straggler that set the load-stream critical path. Matmuls
# run K=126 so the never-written partitions 126/127 are never read.
NP = 126
FULL2, HALF2 = _fw(4), _fw(2)   # DRAM elems/partition per slice
PP_TOTAL = NP * (2 * HALF2 + 7 * FULL2)

_cache = {}


def _build():
    nc = bacc.Bacc(
        "TRN2",
        target_bir_lowering=False,
        debug=False,
        enable_asserts=False,
        num_devices=8,
    )
    # flat pp: [half-slice k=0 (kd 2,3 only)] + [7 full slices] + [half k=8 (kd 0,1)]
    pp_d = nc.dram_tensor(
        "pp", [PP_TOTAL], MM_DT, kind="ExternalInput"
    ).ap()
    wm_d = nc.dram_tensor("wm", [NP, 1024], MM_DT, kind="ExternalInput").ap()
    out_d = nc.dram_tensor(
        "out", [RPC, H, W, C], MM_DT, kind="ExternalOutput"
    ).ap()

    with ExitStack() as ctx:
        tc = ctx.enter_context(tile.TileContext(nc))
        const_pool = ctx.enter_context(tc.tile_pool(name="const", bufs=1))
        # staged slice tiles have exactly one reader (the mega-fold), so
        # slots recycle immediately and a few bufs keep the DMA stream fed
        # without piling up outstanding DMAs (9 outstanding loads measurably
        # degraded early HBM throughput).
        slice_pool = ctx.enter_context(tc.tile_pool(name="slice", bufs=6))
        f_pool = ctx.enter_context(tc.tile_pool(name="fold", bufs=6))
        t_pool = ctx.enter_context(tc.tile_pool(name="tt", bufs=6))
        # every eviction gets its own buffer so all stores can be deferred
        # past the end of the load stream: store DMA packets otherwise
        # interleave with loads on the same SDMA engines and stretch the
        # load-stream critical path by ~5us.
        ev_pool = ctx.enter_context(tc.tile_pool(name="ev", bufs=8))
        psum_pool = ctx.enter_context(tc.tile_pool(name="ps", bufs=4, space="PSUM"))

        # constants go on the scalar-engine HWDGE ring so the sync ring is
        # purely slice loads (HWDGE rings are FIFO per issuing engine).
        wm_sb = const_pool.tile([NP, 1024], MM_DT)
        nc.scalar.dma_start(wm_sb[:], wm_d[:])

        # each slice is staged as two kd-pair PARTS of [NP, 4096] elems:
        # part (k, lo) = kd{0,1}, part (k, hi) = kd{2,3}. Finer load/fold
        # granularity halves the dependent work left after the final DMA
        # byte lands (the drain was fold(7)+combine+MMs+evict+store).
        # Part order in flat pp: (0,hi), (1,lo), (1,hi), ..., (7,lo),
        # (7,hi), (8,lo).
        PFW = 4096

        def load_and_fold(idx):
            t = slice_pool.tile([NP, PFW], MM_DT, tag="slice")
            off = idx * NP * PFW
            # single DMA per part: the splitter spreads 126 partitions over
            # 14 SDMA engines (largest divisor <= 16) at full per-engine
            # rate, and skips engine 15 (the ~20% slower one) entirely.
            # Splitting one tile across two concurrent DMAs (any ring combo,
            # any chunking) halves the per-packet rate -- measured twice,
            # do not try again.
            nc.sync.dma_start(
                t[:], pp_d[off : off + NP * PFW].rearrange("(p f) -> p f", f=PFW)
            )
            # kw-fold of the part in ONE fully contiguous DVE add (vp0 half
            # + vp1 half). Contiguity keeps DVE SBUF-port traffic minimal
            # (strided 8-elem runs waste half of every 32B line and that
            # bank pressure slows DMA/PE under load), and the single reader
            # frees the staged tile immediately for the next load.
            F = f_pool.tile([NP, 2048], MM_DT, tag="F")
            nc.vector.tensor_add(F[:], t[:, 0:2048], t[:, 2048:4096])
            return F

        folds_lo = {}   # kd{0,1} folds, feed rows of their own slice
        folds_hi = {}   # kd{2,3} folds, feed rows of the next slice
        part = 0
        folds_hi[0] = load_and_fold(part)
        part += 1
        for k in range(1, NS):
            folds_lo[k] = load_and_fold(part)
            part += 1
            if k < NS - 1:
                folds_hi[k] = load_and_fold(part)
                part += 1
            # kd-fold for BOTH rows in ONE contiguous 2048-elem DVE add:
            # T2 = [T(row0) | T(row1)] directly (j-major rhs layout).
            T2 = t_pool.tile([NP, 2048], MM_DT, tag="T")
            nc.vector.tensor_add(T2[:], folds_lo[k][:], folds_hi[k - 1][:])
            ev = ev_pool.tile([128, 1024], MM_DT, tag="ev")
            ps = psum_pool.tile([128, 1024], mybir.dt.float32, tag="ps")
            for rr in range(2):
                for half in range(2):
                    outseg = ps[:, rr * 512 + half * 256 : rr * 512 + (half + 1) * 256]
                    for j in range(4):
                        # K=126 (both zero s=63 pad rows dropped from the
                        # transfer); single PE tile position (0,0) as before.
                        lhsT = wm_sb[:, 512 * half + j * 128 : 512 * half + (j + 1) * 128]
                        rhs = T2[:, rr * 1024 + j * 256 : rr * 1024 + (j + 1) * 256]
                        nc.tensor.matmul(
                            outseg, lhsT, rhs, start=(j == 0), stop=(j == 3)
                        )
            # evict on ScalarE: evictions wait on matmuls, and in the DVE
            # FIFO they would delay later folds. rw's interior 0.5 is
            # folded into wm; host rescales the w edges.
            nc.scalar.copy(ev[:], ps[:])
            # stores interleave with loads on the scalar ring; deferring
            # them all past the load stream was measured slower (the tail
            # store drain costs more than the interleave steals).
            d0 = 2 * (k - 1)
            nc.scalar.dma_start(
                out_d[d0 : d0 + 2].rearrange("d h w c -> h d (w c)"),
                ev[:].rearrange("p (d f) -> p d f", d=2),
            )
    nc.compile()
    return nc


def _host_tables():
    rh = np.where(
        (np.arange(H) < 2) | (np.arange(H) >= H - 2), 1.0, 0.5
    ).astype(np.float32)
    # [uhalf*63+s, whalf*512 + j*128 + h], block-diagonal in (uhalf, whalf).
    # 0.25 = interior rd (0.5) * interior rw (0.5); host rescales d/w edges.
    wm = np.zeros((NP, 1024), np.float32)
    s_idx = np.arange(oh)
    for j in range(4):
        h = 2 * s_idx + j
        wm[s_idx, j * 128 + h] = 0.25 * rh[h]
        wm[63 + s_idx, 512 + j * 128 + h] = 0.25 * rh[h]
    return wm.astype(BF16)


def _shard_inputs(patches):
    """Build per-core flat patch blocks. Per slice the layout is
    [p=(uhalf,s)][vpair][kd][j][x=32][t][c] where vpair 0 = kw{0,1} at
    u-slots 1:33 and vpair 1 = kw{2,3} at u-slots 0:32; the two vpair
    halves are contiguous operands of one whole-slice kw-fold add, and
    each folded kd-plane is a contiguous j-major matmul rhs."""
    P5 = np.ascontiguousarray(patches).reshape(B, od, oh, ow, 256).astype(BF16)
    # q-slot k = q+1 for q in [-1, 32); u-slot x = u+1 for u in [-1, 65)
    Pu = np.zeros((B, od + 2, 64, 66, 4, 4, 4, 4), BF16)
    Pu.reshape(B, od + 2, 64, 66, 256)[:, 1 : od + 1, 0:oh, 1 : ow + 1, :] = P5
    pps = []
    for core in range(8):
        b, kc = core // 4, core % 4
        s0 = 8 * kc  # = qbase + 1
        parts = []
        for k in range(NS):
            if k == 0:
                nkd, kdb = 2, 2
            elif k == NS - 1:
                nkd, kdb = 2, 0
            else:
                nkd, kdb = 4, 0
            Q = Pu[b, s0 + k]  # [s=64, u=66, kd, j, v, c]
            blk = np.empty((2, 63, 2, nkd, 4, 32, 2, 4), BF16)
            for uh in range(2):
                for vp in range(2):
                    us = 32 * uh + (1 - vp)
                    # [s, x, kd, j, t, c] -> [s, kd, j, x, t, c]; drop the
                    # all-zero s=63 pad row (partitions are 2*63=126 wide).
                    blk[uh, :, vp] = np.transpose(
                        Q[:63, us : us + 32, kdb : kdb + nkd, :, 2 * vp : 2 * vp + 2, :],
                        (0, 2, 3, 1, 4, 5),
                    )
            if nkd == 2:
                parts.append(blk.reshape(-1))
            else:
                # full slices split into two kd-pair parts: (k, kd01) then
                # (k, kd23), matching the device's part order.
                parts.append(np.ascontiguousarray(blk[:, :, :, 0:2]).reshape(-1))
                parts.append(np.ascontiguousarray(blk[:, :, :, 2:4]).reshape(-1))
        pps.append(np.concatenate(parts))
    return pps


def _run(patches, trace=False):
    if "nc" not in _cache:
        _cache["nc"] = _build()
        _cache["tables"] = _host_tables()
    nc = _cache["nc"]
    wm = _cache["tables"]
    pps = _shard_inputs(np.asarray(patches, dtype=np.float32))
    in_maps = [{"pp": pps[core], "wm": wm} for core in range(8)]
    res = bass_utils.run_bass_kernel_spmd(
        nc, in_maps, core_ids=list(range(8)), trace=trace
    )
    out = np.zeros((B, D, H, W, C), np.float32)
    for core in range(8):
        b, kc = core // 4, core % 4
        out[b, RPC * kc : RPC * (kc + 1)] = np.asarray(
            res.results[core]["out"]
        ).astype(np.float32)
    out[:, [0, 1, D - 2, D - 1]] *= 2.0
    out[:, :, :, [0, 1, W - 2, W - 1], :] *= 2.0
    return out, res


def kernel(patches, inputs):
    out, _ = _run(patches)
    return out

